# revision 39
# baseline (speedup 1.0000x reference)
"""Multi-head attention kernel for Trainium2, batch-parallel across 8 NeuronCores.

Reference (per batch element b, one core each):
  qk = x @ W_qk.T ; q,k = split(qk) ; v = x @ W_v.T
  q,k,v -> [h, n, d] ; q += pos_h ; k += pos_h
  S = q @ k.T * DIM**-0.5 ; mask = outer(m, m) ; masked -> -inf
  P = softmax(S) ; O = P @ v ; out = merge_heads(O) @ W_out.T + b_out

Device strategy (per core):
  - all layout work (transposes, bf16 casts, mask-derived tensors) done on
    HOST in numpy; the device receives x^T, pos^T, W_q^T, W_k^T, W_v^T,
    W_out^T in bf16 and streams pure matmuls.
  - scores computed TRANSPOSED per head pair: the two heads' K=64 score
    matmuls go to row groups 0-1 / 2-3 of the PE array (base partitions 0
    and 64) and run CONCURRENTLY (row tiling).
  - exp via one 1024-wide ACT per (pair, jt, ih) over both heads' scores in
    a [128, 2, 512] PSUM tile; the column mask folds into the per-partition
    exp bias, softmax row sums come from an appended ones-column in the PV
    matmul (V_aug = [V_h | 1], M=65).
  - per-pair attention is two i-half sweeps (ih=0,1) so both heads' PV
    accumulators fit one 2-bank PSUM tile; the softmax tail (1/s, row mask,
    masked-row blend with mean(V)) runs per (ih, head) off the PE path.
  - the next pair's q/k projections are interleaved into the attention
    units so the PE never waits on the ACT engine; projection matmuls are
    kc-major so consecutive matmuls share the stationary operand (hides
    LDWEIGHTS); PV lags the scores by two units.
  - PSUM budget exactly 8 banks: score ring 2x2 + proj 2 + PV acc 2.
  - inputs stream over both hwdge DMA queues (sync + scalar), ordered so
    V-proj dependencies land first; out-projection group 0 accumulates
    kc<7 inside pair 7's filler slots.
"""
import sys

sys.path.insert(0, "/opt/trn_rl_repo")

import numpy as np
import ml_dtypes
from contextlib import ExitStack

B, N, DIM, H = 8, 1024, 1024, 16
D = DIM // H          # 64
E = D + 1             # V_aug block (64 cols of V + ones column)
P = 128
NT = N // P           # 8 n-tiles
KT = DIM // P         # 8 k-tiles
NPAIR = H // 2        # 8 head pairs
SCALE = DIM ** (-0.5)
MB = 30.0             # mask bias magnitude: bias_j = 30*m - 30 in {0, -30}

_NC = None


def _build():
    import concourse.bacc as bacc
    import concourse.bass as bass
    import concourse.mybir as mybir
    import concourse.tile as tile

    f32 = mybir.dt.float32
    bf16 = mybir.dt.bfloat16
    AF = mybir.ActivationFunctionType
    OP = mybir.AluOpType
    ts = bass.ts

    nc = bacc.Bacc()
    xT_d = nc.declare_dram_parameter("xT", [DIM, N], bf16, isOutput=False)
    posT_d = nc.declare_dram_parameter("posT", [DIM, N], bf16, isOutput=False)
    wqT_d = nc.declare_dram_parameter("wqT", [DIM, DIM], bf16, isOutput=False)
    wkT_d = nc.declare_dram_parameter("wkT", [DIM, DIM], bf16, isOutput=False)
    wvT_d = nc.declare_dram_parameter("wvT", [DIM, DIM], bf16, isOutput=False)
    woT_d = nc.declare_dram_parameter("woT", [DIM, DIM], bf16, isOutput=False)
    biasj_d = nc.declare_dram_parameter("biasj", [P, NT], f32, isOutput=False)
    mcoll_d = nc.declare_dram_parameter("mcoll", [P, 16], f32, isOutput=False)
    omm_d = nc.declare_dram_parameter("omm", [N], f32, isOutput=False)
    b_d = nc.declare_dram_parameter("b_out", [DIM], f32, isOutput=False)
    out_d = nc.declare_dram_parameter("out", [N, DIM], f32, isOutput=True)

    with ExitStack() as ctx:
        tc = ctx.enter_context(tile.TileContext(nc))
        sing = ctx.enter_context(tc.tile_pool(name="sing", bufs=1))
        qk_pool = ctx.enter_context(tc.tile_pool(name="qk", bufs=2))
        expool = ctx.enter_context(tc.tile_pool(name="expool", bufs=4))
        tailp = ctx.enter_context(tc.tile_pool(name="tailp", bufs=2))
        ps_st = ctx.enter_context(tc.tile_pool(name="ps_st", bufs=2, space="PSUM"))
        ps_pj = ctx.enter_context(tc.tile_pool(name="ps_pj", bufs=1, space="PSUM"))
        ps_oa = ctx.enter_context(tc.tile_pool(name="ps_oa", bufs=1, space="PSUM"))

        # ---------- persistent SBUF ----------
        xT = sing.tile([P, KT, N], bf16, tag="xT")
        posT = sing.tile([P, KT, N], bf16, tag="posT")
        wqT = sing.tile([P, KT, DIM], bf16, tag="wqT")
        wkT = sing.tile([P, KT, DIM], bf16, tag="wkT")
        wvT = sing.tile([P, KT, DIM], bf16, tag="wvT")
        woT = sing.tile([P, KT, DIM], bf16, tag="woT")
        V_sb = [sing.tile([P, H * E], bf16, tag=f"V{nt}", name=f"V{nt}")
                for nt in range(NT)]
        otfull = [sing.tile([P, N], bf16, tag=f"otf{kc}", name=f"otf{kc}")
                  for kc in range(KT)]
        biasj = sing.tile([P, NT], f32, tag="biasj")
        mcoll = sing.tile([P, 16], f32, tag="mcoll")
        omm_row = sing.tile([1, N], f32, tag="omm_row")
        ommb = sing.tile([D, N], f32, tag="ommb")
        b_row = sing.tile([1, DIM], f32, tag="b_row")
        b_bcast = sing.tile([P, DIM], f32, tag="b_bcast")
        mean_sb = sing.tile([1, D * H], f32, tag="mean_sb")
        mean_cols = sing.tile([D, H], f32, tag="mean_cols")

        # ---------- input DMAs ----------
        # Two parallel hwdge queues (sync + scalar), each ~190 GB/s. Order
        # so V-proj deps (wvT + xT) land first, then pair-0's proj deps.
        # xT loads are per-token-block so V-proj group nt waits only on
        # block nt; wq/wk loads are per-pair slices.
        xTd_v = xT_d.rearrange("(kc p) t -> p kc t", p=P)
        wq_v = wqT_d.rearrange("(kc p) d -> p kc d", p=P)
        wk_v = wkT_d.rearrange("(kc p) d -> p kc d", p=P)
        nc.sync.dma_start(out=xT[:, :, ts(0, P)], in_=xTd_v[:, :, ts(0, P)])
        nc.scalar.dma_start(out=xT[:, :, ts(1, P)], in_=xTd_v[:, :, ts(1, P)])
        for kc in range(0, KT, 2):
            nc.sync.dma_start(out=wvT[:, kc, :], in_=wvT_d[ts(kc, P), :])
        for kc in range(1, KT, 2):
            nc.scalar.dma_start(out=wvT[:, kc, :], in_=wvT_d[ts(kc, P), :])
        for nt in range(2, NT, 2):
            nc.sync.dma_start(out=xT[:, :, ts(nt, P)], in_=xTd_v[:, :, ts(nt, P)])
        for nt in range(3, NT, 2):
            nc.scalar.dma_start(out=xT[:, :, ts(nt, P)], in_=xTd_v[:, :, ts(nt, P)])
        nc.scalar.dma_start(out=biasj, in_=biasj_d[:, :])
        nc.scalar.dma_start(out=mcoll, in_=mcoll_d[:, :])
        nc.scalar.dma_start(out=omm_row, in_=omm_d[:])
        nc.scalar.dma_start(out=b_row, in_=b_d[:])
        for t in range(NPAIR):
            nc.sync.dma_start(out=wqT[:, :, ts(t, P)], in_=wq_v[:, :, ts(t, P)])
            nc.scalar.dma_start(out=wkT[:, :, ts(t, P)], in_=wk_v[:, :, ts(t, P)])
            nc.scalar.dma_start(out=posT[:, t, :], in_=posT_d[ts(t, P), :])
        for kc in range(0, KT, 2):
            nc.sync.dma_start(out=woT[:, kc, :], in_=woT_d[ts(kc, P), :])
        for kc in range(1, KT, 2):
            nc.scalar.dma_start(out=woT[:, kc, :], in_=woT_d[ts(kc, P), :])

        # ---------- PE warm-up during the input DMA wait ----------
        # The HAM clock gate starts at K=4/8 (1.2 GHz) and needs ~3.4us of
        # sustained matmul activity to unthrottle. Burn that time on dummy
        # matmuls over a zeroed scratch tile while the first inputs stream
        # in, so V-proj starts at full clock.
        warm_in = sing.tile([P, 512], bf16, tag="warm_in")
        nc.vector.memset(warm_in, 0.0)
        warm_ps = ps_st.tile([P, 2, 512], f32, tag="st", name="warm_ps")
        for w in range(12):
            nc.tensor.matmul(warm_ps[:, w % 2, :], warm_in[:, 0:P],
                             warm_in, start=True, stop=True)

        # ---------- small prep (off-PE) ----------
        nc.gpsimd.partition_broadcast(ommb, omm_row)
        nc.gpsimd.partition_broadcast(b_bcast, b_row)
        const1 = sing.tile([P, H], f32, tag="const1")
        nc.vector.memset(const1, 1.0)
        for nt in range(NT):
            ones_ap = V_sb[nt].rearrange("p (h e) -> p h e", e=E)[:, :, D:E]
            nc.vector.tensor_copy(ones_ap.squeeze(), const1)
        constN = sing.tile([P, 1], f32, tag="constN")
        nc.vector.memset(constN, 1.0 / N)
        ones_col = sing.tile([P, 1], bf16, tag="ones_col")
        nc.vector.tensor_copy(ones_col, constN)

        # ---------- V = x @ W_v.T  (stored as [V_h | 1] x 16 heads) ----------
        for nt in range(NT):
            pool, tg = (ps_st, "st") if nt % 2 else (ps_pj, "pj")
            pj = pool.tile([P, 2, 512], f32, tag=tg)
            # kc-major so consecutive matmuls share the stationary operand
            for kc in range(KT):
                for dvh in range(2):
                    nc.tensor.matmul(pj[:, dvh, :], xT[:, kc, ts(nt, P)],
                                     wvT[:, kc, ts(dvh, 512)],
                                     start=(kc == 0), stop=(kc == KT - 1))
            for dvh in range(2):
                dst = V_sb[nt][:, dvh * 8 * E: dvh * 8 * E + 8 * E].rearrange(
                    "p (h e) -> p h e", e=E)[:, :, 0:D]
                nc.vector.tensor_copy(
                    dst, pj[:, dvh, :].rearrange("p (h e) -> p h e", e=D))

        # ---------- mean over sequence of V_aug ----------
        # ---------- projection groups (q/k for one pair) ----------
        def proj_ops(t):
            """Returns (ops, results): ops is a list of closures, each emits
            one instruction for the q/k projections of pair t."""
            qT_t = qk_pool.tile([P, N], bf16, tag="qT", name=f"qT{t}")
            kT_t = qk_pool.tile([P, N], bf16, tag="kT", name=f"kT{t}")
            ops = []
            state = {}

            def mk_alloc(which):
                def _op():
                    state[which] = ps_pj.tile([P, 2, 512], f32, tag="pj",
                                              name=f"pj{which}")
                return _op

            def mk_mm(which, w_sb, half, kc):
                def _op():
                    nc.tensor.matmul(state[which][:, half, :],
                                     w_sb[:, kc, ts(t, P)],
                                     xT[:, kc, ts(half, 512)],
                                     start=(kc == 0), stop=(kc == KT - 1))
                return _op

            def mk_tt(which, dstT, half):
                def _op():
                    nc.vector.tensor_add(dstT[:, ts(half, 512)],
                                         state[which][:, half, :],
                                         posT[:, t, ts(half, 512)])
                return _op

            for which, w_sb, dstT in (("q", wqT, qT_t), ("k", wkT, kT_t)):
                ops.append(mk_alloc(which))
                # kc-major: both halves reuse the same stationary weights
                for kc in range(KT):
                    for half in range(2):
                        ops.append(mk_mm(which, w_sb, half, kc))
                for half in range(2):
                    ops.append(mk_tt(which, dstT, half))
            return ops, (qT_t, kT_t)

        # pair-0 projections first (only need xT; mean needs V_sb copies)
        ops0, qk0 = proj_ops(0)
        for op in ops0:
            op()

        # head-aligned chunks (7h, 7h, 2h); psum->sbuf copies write mean_sb
        # in (e, h) order so one flat DMA yields mean_cols[e, h].
        mt = ps_pj.tile([P, 2, 512], f32, tag="pj")
        mt2 = ps_st.tile([P, 2, 512], f32, tag="st")
        chunks = ((0, 7, mt[0:1, 0, :]), (7, 7, mt[0:1, 1, :]),
                  (14, 2, mt2[0:1, 0, :]))
        for h0, hn, dstp in chunks:
            for nt in range(NT):
                nc.tensor.matmul(dstp[:, 0:hn * E], ones_col,
                                 V_sb[nt][:, h0 * E:(h0 + hn) * E],
                                 start=(nt == 0), stop=(nt == NT - 1))
        mean_eh = mean_sb.rearrange("o (e h) -> o e h", h=H)  # [1, 64, 16]
        for h0, hn, dstp in chunks:
            nc.vector.tensor_copy(
                mean_eh[:, :, h0:h0 + hn].rearrange("o e h -> o h e"),
                dstp[:, 0:hn * E].rearrange("o (h e) -> o h e", e=E)[:, :, 0:D])
        nc.sync.dma_start(out=mean_cols, in_=mean_sb[0:1, 0:D * H])

        # out-projection group 0, kc 0-6: fed as pair-7 sweep-1 fillers
        oproj_state = {}

        def oproj0_partial_ops():
            ops = []

            def alloc():
                oproj_state["pj"] = ps_pj.tile([P, 2, 512], f32, tag="pj",
                                               name="opj0")
            ops.append(alloc)

            def mk(kc, doh):
                def _op():
                    nc.tensor.matmul(oproj_state["pj"][:, doh, :],
                                     otfull[kc][:, ts(0, P)],
                                     woT[:, kc, ts(doh, 512)],
                                     start=(kc == 0), stop=False)
                return _op

            for kc in range(KT - 1):
                for doh in range(2):
                    ops.append(mk(kc, doh))
            return ops

        # ---------- per-pair attention ----------
        cur_qk = qk0
        for t in range(NPAIR):
            qT_t, kT_t = cur_qk
            if t + 1 < NPAIR:
                pend, cur_qk = proj_ops(t + 1)
            else:
                pend, cur_qk = oproj0_partial_ops(), None
            pend = list(pend)

            for ih in range(2):
                oaf = ps_oa.tile([P, 2, 512], f32, tag="oa", name=f"oa{t}_{ih}")
                oa = oaf[0:E, :, :]
                exs = [None] * NT

                def emit_pv(jt):
                    for hs in range(2):
                        h = 2 * t + hs
                        nc.tensor.matmul(oa[:, hs, :],
                                         V_sb[jt][:, h * E:(h + 1) * E],
                                         exs[jt][:, hs, :],
                                         start=(jt == 0), stop=(jt == NT - 1))

                for jt in range(NT):
                    st = ps_st.tile([P, 2, 512], f32, tag="st")
                    for hs in range(2):
                        nc.tensor.matmul(st[:, hs, :],
                                         kT_t[ts(hs, D), ts(jt, P)],
                                         qT_t[ts(hs, D), ts(ih, 512)],
                                         start=True, stop=True)
                    ex = expool.tile([P, 2, 512], bf16, tag="ex")
                    nc.scalar.activation(ex, st, AF.Exp,
                                         bias=biasj[:, jt:jt + 1], scale=SCALE)
                    exs[jt] = ex
                    # PV lags one jt behind (two at sweep start, giving the
                    # oaS drain of the previous sweep room to free the acc).
                    if jt == 1:
                        pass
                    elif jt > 1:
                        emit_pv(jt - 2)
                    # front-loaded so the proj TTs land well before the
                    # next pair's score matmuls need qT/kT. Pair 7's
                    # fillers (out-proj kc<7) wait on pair-6 tails, so
                    # only feed them in sweep ih=1.
                    if t < NPAIR - 1 or ih == 1:
                        for _ in range(3):
                            if pend:
                                pend.pop(0)()
                emit_pv(NT - 2)
                emit_pv(NT - 1)

                # ---- softmax tail for (t, ih), both heads ----
                oaS = tailp.tile([E, 2, 512], f32, tag="oaS")
                nc.vector.tensor_copy(oaS, oa)     # frees the PSUM acc
                # collect s rows into [p, c] layout: s_coll[p, hs, c] =
                # s_hs[p*4 + c] (both APs flatten row-major -> streams match)
                s_coll = tailp.tile([P, 2, 4], f32, tag="s_coll")
                for hs in range(2):
                    nc.sync.dma_start(out=s_coll[:, hs, :],
                                      in_=oaS[D:D + 1, hs, :])
                r_coll = tailp.tile([P, 2, 4], f32, tag="r_coll")
                nc.vector.reciprocal(r_coll, s_coll)
                nc.vector.tensor_mul(
                    r_coll, r_coll,
                    mcoll[:, ih * 8:(ih + 1) * 8].rearrange(
                        "p (h c) -> p h c", c=4))
                for hs in range(2):
                    h = 2 * t + hs
                    rm_row = tailp.tile([1, 512], f32, tag=f"rm{hs}")
                    nc.sync.dma_start(
                        out=rm_row.rearrange("o (p c) -> o p c", c=4),
                        in_=r_coll[:, hs, :],
                    )
                    rmb = tailp.tile([D, 512], f32, tag=f"rmb{hs}")
                    nc.gpsimd.partition_broadcast(rmb, rm_row)
                    t1 = tailp.tile([D, 512], f32, tag=f"t1{hs}")
                    nc.vector.tensor_mul(t1, oaS[0:D, hs, :], rmb)
                    if hs == 0:
                        nc.vector.scalar_tensor_tensor(
                            otfull[t][0:D, ts(ih, 512)],
                            ommb[:, ts(ih, 512)], mean_cols[:, h:h + 1], t1,
                            OP.mult, OP.add)
                    else:
                        hscr = tailp.tile([D, 512], bf16, tag="hscr")
                        nc.vector.scalar_tensor_tensor(
                            hscr, ommb[:, ts(ih, 512)],
                            mean_cols[:, h:h + 1], t1, OP.mult, OP.add)
                        nc.sync.dma_start(
                            out=otfull[t][D:P, ts(ih, 512)], in_=hscr)
            # any leftover projection ops
            for op in pend:
                op()

        # ---------- out projection (3 psum rings: st, oa, pj) ----------
        def _store_out(nt, doh, pj):
            ostage = tailp.tile([P, 512], f32, tag="ostage", bufs=3)
            nc.vector.tensor_add(ostage, pj[:, doh, :],
                                 b_bcast[:, ts(doh, 512)])
            eng = nc.sync if (nt + doh) % 2 == 0 else nc.scalar
            eng.dma_start(out=out_d[ts(nt, P), ts(doh, 512)], in_=ostage)

        for nt in range(NT):
            if nt == 0:
                # finish the group started as pair-7 fillers
                pj = oproj_state["pj"]
                for doh in range(2):
                    nc.tensor.matmul(pj[:, doh, :],
                                     otfull[KT - 1][:, ts(0, P)],
                                     woT[:, KT - 1, ts(doh, 512)],
                                     start=False, stop=True)
            else:
                pool, tg = ((ps_st, "st"), (ps_oa, "oa"),
                            (ps_pj, "pj"))[nt % 3]
                pj = pool.tile([P, 2, 512], f32, tag=tg)
                doh_major = nt == NT - 1  # last group: drain doh=0 early
                for doh in range(2):
                    if doh_major:
                        for kc in range(KT):
                            nc.tensor.matmul(
                                pj[:, doh, :], otfull[kc][:, ts(nt, P)],
                                woT[:, kc, ts(doh, 512)],
                                start=(kc == 0), stop=(kc == KT - 1))
                        _store_out(nt, doh, pj)
                if not doh_major:
                    for kc in range(KT):
                        for doh in range(2):
                            nc.tensor.matmul(
                                pj[:, doh, :], otfull[kc][:, ts(nt, P)],
                                woT[:, kc, ts(doh, 512)],
                                start=(kc == 0), stop=(kc == KT - 1))
            if nt != NT - 1:
                for doh in range(2):
                    _store_out(nt, doh, pj)

    nc.finalize()
    return nc


def _host_prep(x, mask, pos, W_qk, W_v, W_out, b_out):
    bf = ml_dtypes.bfloat16
    x = np.ascontiguousarray(x, dtype=np.float32)
    pos = np.ascontiguousarray(pos, dtype=np.float32)
    W_qk = np.asarray(W_qk, dtype=np.float32)
    maskf = np.concatenate(
        [np.ones((B, 1), np.float32), np.asarray(mask).astype(np.float32)],
        axis=1)                                        # [B, N]
    wqT = np.ascontiguousarray(W_qk[:DIM].T.astype(bf))
    wkT = np.ascontiguousarray(W_qk[DIM:].T.astype(bf))
    wvT = np.ascontiguousarray(np.asarray(W_v, np.float32).T.astype(bf))
    woT = np.ascontiguousarray(np.asarray(W_out, np.float32).T.astype(bf))
    b_out = np.ascontiguousarray(b_out, dtype=np.float32)

    in_maps = []
    for b in range(B):
        m = maskf[b]
        biasj = np.ascontiguousarray(
            (MB * m - MB).reshape(NT, P).T)            # [p, jt]
        # mcoll[p, ih*8 + hs*4 + c] = m[ih*512 + p*4 + c] (dup for both heads)
        mc = m.reshape(2, P, 4)                        # [ih, p, c]
        mcoll = np.ascontiguousarray(
            np.stack([mc[0], mc[0], mc[1], mc[1]],
                     axis=1).reshape(P, 16))
        in_maps.append({
            "xT": np.ascontiguousarray(x[b].T.astype(bf)),
            "posT": np.ascontiguousarray(pos[b].T.astype(bf)),
            "wqT": wqT, "wkT": wkT, "wvT": wvT, "woT": woT,
            "biasj": biasj.astype(np.float32),
            "mcoll": mcoll.astype(np.float32),
            "omm": np.ascontiguousarray(1.0 - m),
            "b_out": b_out,
        })
    return in_maps


def kernel(x, mask, pos, W_qk, W_v, W_out, b_out):
    global _NC
    from concourse.bass_utils import run_bass_kernel_spmd

    if _NC is None:
        _NC = _build()

    in_maps = _host_prep(x, mask, pos, W_qk, W_v, W_out, b_out)
    try:
        res = run_bass_kernel_spmd(
            _NC, [dict(m) for m in in_maps], core_ids=list(range(B)))
    except Exception:
        # a previously-wedged device can fail the first execution; a retry
        # on the recovered device succeeds
        res = run_bass_kernel_spmd(_NC, in_maps, core_ids=list(range(B)))
    return np.stack([res.results[b]["out"] for b in range(B)]).astype(np.float32)


# revision 42
# speedup vs baseline: 1.0035x; 1.0035x over previous
"""Multi-head attention kernel for Trainium2, batch-parallel across 8 NeuronCores.

Reference (per batch element b, one core each):
  qk = x @ W_qk.T ; q,k = split(qk) ; v = x @ W_v.T
  q,k,v -> [h, n, d] ; q += pos_h ; k += pos_h
  S = q @ k.T * DIM**-0.5 ; mask = outer(m, m) ; masked -> -inf
  P = softmax(S) ; O = P @ v ; out = merge_heads(O) @ W_out.T + b_out

Device strategy (per core):
  - all layout work (transposes, bf16 casts, mask-derived tensors) done on
    HOST in numpy; the device receives x^T, pos^T, W_q^T, W_k^T, W_v^T,
    W_out^T in bf16 and streams pure matmuls.
  - scores computed TRANSPOSED per head pair: the two heads' K=64 score
    matmuls go to row groups 0-1 / 2-3 of the PE array (base partitions 0
    and 64) and run CONCURRENTLY (row tiling).
  - exp via one 1024-wide ACT per (pair, jt, ih) over both heads' scores in
    a [128, 2, 512] PSUM tile; the column mask folds into the per-partition
    exp bias, softmax row sums come from an appended ones-column in the PV
    matmul (V_aug = [V_h | 1], M=65).
  - per-pair attention is two i-half sweeps (ih=0,1) so both heads' PV
    accumulators fit one 2-bank PSUM tile; the softmax tail (1/s, row mask,
    masked-row blend with mean(V)) runs per (ih, head) off the PE path.
  - the next pair's q/k projections are interleaved into the attention
    units so the PE never waits on the ACT engine; projection matmuls are
    kc-major so consecutive matmuls share the stationary operand (hides
    LDWEIGHTS); PV lags the scores by two units.
  - PSUM budget exactly 8 banks: score ring 2x2 + proj 2 + PV acc 2.
  - inputs stream over both hwdge DMA queues (sync + scalar), ordered so
    V-proj dependencies land first; out-projection group 0 accumulates
    kc<7 inside pair 7's filler slots.
"""
import sys

sys.path.insert(0, "/opt/trn_rl_repo")

import numpy as np
import ml_dtypes
from contextlib import ExitStack

B, N, DIM, H = 8, 1024, 1024, 16
D = DIM // H          # 64
E = D + 1             # V_aug block (64 cols of V + ones column)
P = 128
NT = N // P           # 8 n-tiles
KT = DIM // P         # 8 k-tiles
NPAIR = H // 2        # 8 head pairs
SCALE = DIM ** (-0.5)
MB = 30.0             # mask bias magnitude: bias_j = 30*m - 30 in {0, -30}

_NC = None


def _build():
    import concourse.bacc as bacc
    import concourse.bass as bass
    import concourse.mybir as mybir
    import concourse.tile as tile

    f32 = mybir.dt.float32
    bf16 = mybir.dt.bfloat16
    AF = mybir.ActivationFunctionType
    OP = mybir.AluOpType
    ts = bass.ts

    nc = bacc.Bacc()
    xT_d = nc.declare_dram_parameter("xT", [DIM, N], bf16, isOutput=False)
    posT_d = nc.declare_dram_parameter("posT", [DIM, N], bf16, isOutput=False)
    wqT_d = nc.declare_dram_parameter("wqT", [DIM, DIM], bf16, isOutput=False)
    wkT_d = nc.declare_dram_parameter("wkT", [DIM, DIM], bf16, isOutput=False)
    wvT_d = nc.declare_dram_parameter("wvT", [DIM, DIM], bf16, isOutput=False)
    woT_d = nc.declare_dram_parameter("woT", [DIM, DIM], bf16, isOutput=False)
    biasj_d = nc.declare_dram_parameter("biasj", [P, NT], f32, isOutput=False)
    mcoll_d = nc.declare_dram_parameter("mcoll", [P, 16], f32, isOutput=False)
    omm_d = nc.declare_dram_parameter("omm", [N], f32, isOutput=False)
    b_d = nc.declare_dram_parameter("b_out", [DIM], f32, isOutput=False)
    out_d = nc.declare_dram_parameter("out", [N, DIM], f32, isOutput=True)

    with ExitStack() as ctx:
        tc = ctx.enter_context(tile.TileContext(nc))
        sing = ctx.enter_context(tc.tile_pool(name="sing", bufs=1))
        qk_pool = ctx.enter_context(tc.tile_pool(name="qk", bufs=2))
        expool = ctx.enter_context(tc.tile_pool(name="expool", bufs=4))
        tailp = ctx.enter_context(tc.tile_pool(name="tailp", bufs=2))
        ps_st = ctx.enter_context(tc.tile_pool(name="ps_st", bufs=2, space="PSUM"))
        ps_pj = ctx.enter_context(tc.tile_pool(name="ps_pj", bufs=1, space="PSUM"))
        ps_oa = ctx.enter_context(tc.tile_pool(name="ps_oa", bufs=1, space="PSUM"))

        # ---------- persistent SBUF ----------
        xT = sing.tile([P, KT, N], bf16, tag="xT")
        posT = sing.tile([P, KT, N], bf16, tag="posT")
        wqT = sing.tile([P, KT, DIM], bf16, tag="wqT")
        wkT = sing.tile([P, KT, DIM], bf16, tag="wkT")
        wvT = sing.tile([P, KT, DIM], bf16, tag="wvT")
        woT = sing.tile([P, KT, DIM], bf16, tag="woT")
        V_sb = [sing.tile([P, H * E], bf16, tag=f"V{nt}", name=f"V{nt}")
                for nt in range(NT)]
        otfull = [sing.tile([P, N], bf16, tag=f"otf{kc}", name=f"otf{kc}")
                  for kc in range(KT)]
        biasj = sing.tile([P, NT], f32, tag="biasj")
        mcoll = sing.tile([P, 16], f32, tag="mcoll")
        omm_row = sing.tile([1, N], f32, tag="omm_row")
        ommb = sing.tile([D, N], f32, tag="ommb")
        b_row = sing.tile([1, DIM], f32, tag="b_row")
        b_bcast = sing.tile([P, DIM], f32, tag="b_bcast")
        mean_sb = sing.tile([1, D * H], f32, tag="mean_sb")
        mean_cols = sing.tile([D, H], f32, tag="mean_cols")

        # ---------- input DMAs ----------
        # Two parallel hwdge queues (sync + scalar), each ~190 GB/s. Order
        # so V-proj deps (wvT + xT) land first, then pair-0's proj deps.
        # xT loads are per-token-block so V-proj group nt waits only on
        # block nt; wq/wk loads are per-pair slices.
        xTd_v = xT_d.rearrange("(kc p) t -> p kc t", p=P)
        wq_v = wqT_d.rearrange("(kc p) d -> p kc d", p=P)
        wk_v = wkT_d.rearrange("(kc p) d -> p kc d", p=P)
        nc.sync.dma_start(out=xT[:, :, ts(0, P)], in_=xTd_v[:, :, ts(0, P)])
        nc.scalar.dma_start(out=xT[:, :, ts(1, P)], in_=xTd_v[:, :, ts(1, P)])
        for kc in range(0, KT, 2):
            nc.sync.dma_start(out=wvT[:, kc, :], in_=wvT_d[ts(kc, P), :])
        for kc in range(1, KT, 2):
            nc.scalar.dma_start(out=wvT[:, kc, :], in_=wvT_d[ts(kc, P), :])
        for nt in range(2, NT, 2):
            nc.sync.dma_start(out=xT[:, :, ts(nt, P)], in_=xTd_v[:, :, ts(nt, P)])
        for nt in range(3, NT, 2):
            nc.scalar.dma_start(out=xT[:, :, ts(nt, P)], in_=xTd_v[:, :, ts(nt, P)])
        nc.scalar.dma_start(out=biasj, in_=biasj_d[:, :])
        nc.scalar.dma_start(out=mcoll, in_=mcoll_d[:, :])
        nc.scalar.dma_start(out=omm_row, in_=omm_d[:])
        nc.scalar.dma_start(out=b_row, in_=b_d[:])
        for t in range(NPAIR):
            nc.sync.dma_start(out=wqT[:, :, ts(t, P)], in_=wq_v[:, :, ts(t, P)])
            nc.scalar.dma_start(out=wkT[:, :, ts(t, P)], in_=wk_v[:, :, ts(t, P)])
            nc.scalar.dma_start(out=posT[:, t, :], in_=posT_d[ts(t, P), :])
        for kc in range(0, KT, 2):
            nc.sync.dma_start(out=woT[:, kc, :], in_=woT_d[ts(kc, P), :])
        for kc in range(1, KT, 2):
            nc.scalar.dma_start(out=woT[:, kc, :], in_=woT_d[ts(kc, P), :])

        # ---------- PE warm-up during the input DMA wait ----------
        # The HAM clock gate starts at K=4/8 (1.2 GHz) and needs ~3.4us of
        # sustained matmul activity to unthrottle. Burn that time on dummy
        # matmuls over a zeroed scratch tile while the first inputs stream
        # in, so V-proj starts at full clock.
        warm_in = sing.tile([P, 512], bf16, tag="warm_in")
        nc.vector.memset(warm_in, 0.0)
        warm_ps = ps_st.tile([P, 2, 512], f32, tag="st", name="warm_ps")
        for w in range(12):
            nc.tensor.matmul(warm_ps[:, w % 2, :], warm_in[:, 0:P],
                             warm_in, start=True, stop=True)

        # ---------- small prep (off-PE) ----------
        nc.gpsimd.partition_broadcast(ommb, omm_row)
        nc.gpsimd.partition_broadcast(b_bcast, b_row)
        const1 = sing.tile([P, H], f32, tag="const1")
        nc.vector.memset(const1, 1.0)
        for nt in range(NT):
            ones_ap = V_sb[nt].rearrange("p (h e) -> p h e", e=E)[:, :, D:E]
            nc.vector.tensor_copy(ones_ap.squeeze(), const1)
        constN = sing.tile([P, 1], f32, tag="constN")
        nc.vector.memset(constN, 1.0 / N)
        ones_col = sing.tile([P, 1], bf16, tag="ones_col")
        nc.vector.tensor_copy(ones_col, constN)

        # ---------- V = x @ W_v.T  (stored as [V_h | 1] x 16 heads) ----------
        def emit_v_group(nt):
            pool, tg = (ps_st, "st") if nt % 2 else (ps_pj, "pj")
            pj = pool.tile([P, 2, 512], f32, tag=tg)
            # kc-major so consecutive matmuls share the stationary operand
            for kc in range(KT):
                for dvh in range(2):
                    nc.tensor.matmul(pj[:, dvh, :], xT[:, kc, ts(nt, P)],
                                     wvT[:, kc, ts(dvh, 512)],
                                     start=(kc == 0), stop=(kc == KT - 1))
            for dvh in range(2):
                dst = V_sb[nt][:, dvh * 8 * E: dvh * 8 * E + 8 * E].rearrange(
                    "p (h e) -> p h e", e=E)[:, :, 0:D]
                nc.vector.tensor_copy(
                    dst, pj[:, dvh, :].rearrange("p (h e) -> p h e", e=D))

        for nt in range(NT - 1):
            emit_v_group(nt)

        # ---------- mean over sequence of V_aug ----------
        # ---------- projection groups (q/k for one pair) ----------
        def proj_ops(t):
            """Returns (ops, results): ops is a list of closures, each emits
            one instruction for the q/k projections of pair t."""
            qT_t = qk_pool.tile([P, N], bf16, tag="qT", name=f"qT{t}")
            kT_t = qk_pool.tile([P, N], bf16, tag="kT", name=f"kT{t}")
            ops = []
            state = {}

            def mk_alloc(which):
                def _op():
                    state[which] = ps_pj.tile([P, 2, 512], f32, tag="pj",
                                              name=f"pj{which}")
                return _op

            def mk_mm(which, w_sb, half, kc):
                def _op():
                    nc.tensor.matmul(state[which][:, half, :],
                                     w_sb[:, kc, ts(t, P)],
                                     xT[:, kc, ts(half, 512)],
                                     start=(kc == 0), stop=(kc == KT - 1))
                return _op

            def mk_tt(which, dstT, half):
                def _op():
                    nc.vector.tensor_add(dstT[:, ts(half, 512)],
                                         state[which][:, half, :],
                                         posT[:, t, ts(half, 512)])
                return _op

            for which, w_sb, dstT in (("q", wqT, qT_t), ("k", wkT, kT_t)):
                ops.append(mk_alloc(which))
                # kc-major: both halves reuse the same stationary weights
                for kc in range(KT):
                    for half in range(2):
                        ops.append(mk_mm(which, w_sb, half, kc))
                for half in range(2):
                    ops.append(mk_tt(which, dstT, half))
            return ops, (qT_t, kT_t)

        # pair-0 projections slotted before the last V group so the pj-ring
        # drain overlaps and qT0/kT0 land earlier
        ops0, qk0 = proj_ops(0)
        for op in ops0:
            op()
        emit_v_group(NT - 1)

        # head-aligned chunks (7h, 7h, 2h); psum->sbuf copies write mean_sb
        # in (e, h) order so one flat DMA yields mean_cols[e, h].
        mt = ps_pj.tile([P, 2, 512], f32, tag="pj")
        mt2 = ps_st.tile([P, 2, 512], f32, tag="st")
        chunks = ((0, 7, mt[0:1, 0, :]), (7, 7, mt[0:1, 1, :]),
                  (14, 2, mt2[0:1, 0, :]))
        for h0, hn, dstp in chunks:
            for nt in range(NT):
                nc.tensor.matmul(dstp[:, 0:hn * E], ones_col,
                                 V_sb[nt][:, h0 * E:(h0 + hn) * E],
                                 start=(nt == 0), stop=(nt == NT - 1))
        mean_eh = mean_sb.rearrange("o (e h) -> o e h", h=H)  # [1, 64, 16]
        for h0, hn, dstp in chunks:
            nc.vector.tensor_copy(
                mean_eh[:, :, h0:h0 + hn].rearrange("o e h -> o h e"),
                dstp[:, 0:hn * E].rearrange("o (h e) -> o h e", e=E)[:, :, 0:D])
        nc.sync.dma_start(out=mean_cols, in_=mean_sb[0:1, 0:D * H])

        # out-projection group 0, kc 0-6: fed as pair-7 sweep-1 fillers
        oproj_state = {}

        def oproj0_partial_ops():
            ops = []

            def alloc():
                oproj_state["pj"] = ps_pj.tile([P, 2, 512], f32, tag="pj",
                                               name="opj0")
            ops.append(alloc)

            def mk(kc, doh):
                def _op():
                    nc.tensor.matmul(oproj_state["pj"][:, doh, :],
                                     otfull[kc][:, ts(0, P)],
                                     woT[:, kc, ts(doh, 512)],
                                     start=(kc == 0), stop=False)
                return _op

            for kc in range(KT - 1):
                for doh in range(2):
                    ops.append(mk(kc, doh))
            return ops

        # ---------- per-pair attention ----------
        cur_qk = qk0
        for t in range(NPAIR):
            qT_t, kT_t = cur_qk
            if t + 1 < NPAIR:
                pend, cur_qk = proj_ops(t + 1)
            else:
                pend, cur_qk = oproj0_partial_ops(), None
            pend = list(pend)

            for ih in range(2):
                oaf = ps_oa.tile([P, 2, 512], f32, tag="oa", name=f"oa{t}_{ih}")
                oa = oaf[0:E, :, :]
                exs = [None] * NT

                def emit_pv(jt):
                    for hs in range(2):
                        h = 2 * t + hs
                        nc.tensor.matmul(oa[:, hs, :],
                                         V_sb[jt][:, h * E:(h + 1) * E],
                                         exs[jt][:, hs, :],
                                         start=(jt == 0), stop=(jt == NT - 1))

                for jt in range(NT):
                    st = ps_st.tile([P, 2, 512], f32, tag="st")
                    for hs in range(2):
                        nc.tensor.matmul(st[:, hs, :],
                                         kT_t[ts(hs, D), ts(jt, P)],
                                         qT_t[ts(hs, D), ts(ih, 512)],
                                         start=True, stop=True)
                    ex = expool.tile([P, 2, 512], bf16, tag="ex")
                    nc.scalar.activation(ex, st, AF.Exp,
                                         bias=biasj[:, jt:jt + 1], scale=SCALE)
                    exs[jt] = ex
                    # PV lags one jt behind (two at sweep start, giving the
                    # oaS drain of the previous sweep room to free the acc).
                    if jt == 1:
                        pass
                    elif jt > 1:
                        emit_pv(jt - 2)
                    # front-loaded so the proj TTs land well before the
                    # next pair's score matmuls need qT/kT. Pair 7's
                    # fillers (out-proj kc<7) wait on pair-6 tails, so
                    # only feed them in sweep ih=1.
                    if t < NPAIR - 1 or ih == 1:
                        for _ in range(3):
                            if pend:
                                pend.pop(0)()
                emit_pv(NT - 2)
                emit_pv(NT - 1)

                # ---- softmax tail for (t, ih), both heads ----
                oaS = tailp.tile([E, 2, 512], f32, tag="oaS")
                nc.vector.tensor_copy(oaS, oa)     # frees the PSUM acc
                # collect s rows into [p, c] layout: s_coll[p, hs, c] =
                # s_hs[p*4 + c] (both APs flatten row-major -> streams match)
                s_coll = tailp.tile([P, 2, 4], f32, tag="s_coll")
                for hs in range(2):
                    nc.sync.dma_start(out=s_coll[:, hs, :],
                                      in_=oaS[D:D + 1, hs, :])
                r_coll = tailp.tile([P, 2, 4], f32, tag="r_coll")
                nc.vector.reciprocal(r_coll, s_coll)
                nc.vector.tensor_mul(
                    r_coll, r_coll,
                    mcoll[:, ih * 8:(ih + 1) * 8].rearrange(
                        "p (h c) -> p h c", c=4))
                for hs in range(2):
                    h = 2 * t + hs
                    rm_row = tailp.tile([1, 512], f32, tag=f"rm{hs}")
                    nc.sync.dma_start(
                        out=rm_row.rearrange("o (p c) -> o p c", c=4),
                        in_=r_coll[:, hs, :],
                    )
                    rmb = tailp.tile([D, 512], f32, tag=f"rmb{hs}")
                    nc.gpsimd.partition_broadcast(rmb, rm_row)
                    t1 = tailp.tile([D, 512], f32, tag=f"t1{hs}")
                    nc.vector.tensor_mul(t1, oaS[0:D, hs, :], rmb)
                    if hs == 0:
                        nc.vector.scalar_tensor_tensor(
                            otfull[t][0:D, ts(ih, 512)],
                            ommb[:, ts(ih, 512)], mean_cols[:, h:h + 1], t1,
                            OP.mult, OP.add)
                    else:
                        hscr = tailp.tile([D, 512], bf16, tag="hscr")
                        nc.vector.scalar_tensor_tensor(
                            hscr, ommb[:, ts(ih, 512)],
                            mean_cols[:, h:h + 1], t1, OP.mult, OP.add)
                        nc.sync.dma_start(
                            out=otfull[t][D:P, ts(ih, 512)], in_=hscr)
            # any leftover projection ops
            for op in pend:
                op()

        # ---------- out projection (3 psum rings: st, oa, pj) ----------
        def _store_out(nt, doh, pj):
            ostage = tailp.tile([P, 512], f32, tag="ostage", bufs=3)
            nc.vector.tensor_add(ostage, pj[:, doh, :],
                                 b_bcast[:, ts(doh, 512)])
            if nt == NT - 1 and doh == 1:
                # final store: split across both queues to halve the flight
                nc.sync.dma_start(
                    out=out_d[nt * P:nt * P + D, ts(doh, 512)],
                    in_=ostage[0:D, :])
                nc.scalar.dma_start(
                    out=out_d[nt * P + D:(nt + 1) * P, ts(doh, 512)],
                    in_=ostage[D:P, :])
            else:
                eng = nc.sync if (nt + doh) % 2 == 0 else nc.scalar
                eng.dma_start(out=out_d[ts(nt, P), ts(doh, 512)], in_=ostage)

        for nt in range(NT):
            if nt == 0:
                # finish the group started as pair-7 fillers
                pj = oproj_state["pj"]
                for doh in range(2):
                    nc.tensor.matmul(pj[:, doh, :],
                                     otfull[KT - 1][:, ts(0, P)],
                                     woT[:, KT - 1, ts(doh, 512)],
                                     start=False, stop=True)
            else:
                pool, tg = ((ps_st, "st"), (ps_oa, "oa"),
                            (ps_pj, "pj"))[nt % 3]
                pj = pool.tile([P, 2, 512], f32, tag=tg)
                doh_major = nt == NT - 1  # last group: drain doh=0 early
                for doh in range(2):
                    if doh_major:
                        for kc in range(KT):
                            nc.tensor.matmul(
                                pj[:, doh, :], otfull[kc][:, ts(nt, P)],
                                woT[:, kc, ts(doh, 512)],
                                start=(kc == 0), stop=(kc == KT - 1))
                        _store_out(nt, doh, pj)
                if not doh_major:
                    for kc in range(KT):
                        for doh in range(2):
                            nc.tensor.matmul(
                                pj[:, doh, :], otfull[kc][:, ts(nt, P)],
                                woT[:, kc, ts(doh, 512)],
                                start=(kc == 0), stop=(kc == KT - 1))
            if nt != NT - 1:
                for doh in range(2):
                    _store_out(nt, doh, pj)

    nc.finalize()
    return nc


def _host_prep(x, mask, pos, W_qk, W_v, W_out, b_out):
    bf = ml_dtypes.bfloat16
    x = np.ascontiguousarray(x, dtype=np.float32)
    pos = np.ascontiguousarray(pos, dtype=np.float32)
    W_qk = np.asarray(W_qk, dtype=np.float32)
    maskf = np.concatenate(
        [np.ones((B, 1), np.float32), np.asarray(mask).astype(np.float32)],
        axis=1)                                        # [B, N]
    wqT = np.ascontiguousarray(W_qk[:DIM].T.astype(bf))
    wkT = np.ascontiguousarray(W_qk[DIM:].T.astype(bf))
    wvT = np.ascontiguousarray(np.asarray(W_v, np.float32).T.astype(bf))
    woT = np.ascontiguousarray(np.asarray(W_out, np.float32).T.astype(bf))
    b_out = np.ascontiguousarray(b_out, dtype=np.float32)

    in_maps = []
    for b in range(B):
        m = maskf[b]
        biasj = np.ascontiguousarray(
            (MB * m - MB).reshape(NT, P).T)            # [p, jt]
        # mcoll[p, ih*8 + hs*4 + c] = m[ih*512 + p*4 + c] (dup for both heads)
        mc = m.reshape(2, P, 4)                        # [ih, p, c]
        mcoll = np.ascontiguousarray(
            np.stack([mc[0], mc[0], mc[1], mc[1]],
                     axis=1).reshape(P, 16))
        in_maps.append({
            "xT": np.ascontiguousarray(x[b].T.astype(bf)),
            "posT": np.ascontiguousarray(pos[b].T.astype(bf)),
            "wqT": wqT, "wkT": wkT, "wvT": wvT, "woT": woT,
            "biasj": biasj.astype(np.float32),
            "mcoll": mcoll.astype(np.float32),
            "omm": np.ascontiguousarray(1.0 - m),
            "b_out": b_out,
        })
    return in_maps


def kernel(x, mask, pos, W_qk, W_v, W_out, b_out):
    global _NC
    from concourse.bass_utils import run_bass_kernel_spmd

    if _NC is None:
        _NC = _build()

    in_maps = _host_prep(x, mask, pos, W_qk, W_v, W_out, b_out)
    try:
        res = run_bass_kernel_spmd(
            _NC, [dict(m) for m in in_maps], core_ids=list(range(B)))
    except Exception:
        # a previously-wedged device can fail the first execution; a retry
        # on the recovered device succeeds
        res = run_bass_kernel_spmd(_NC, in_maps, core_ids=list(range(B)))
    return np.stack([res.results[b]["out"] for b in range(B)]).astype(np.float32)
